# revision 4
# baseline (speedup 1.0000x reference)
"""CoAttenBlock Trainium2 kernel (B=4, C=32, H=W=64, N=4096), 8 NeuronCores.

Sharding: core k -> batch b=k//2, i-half h=k%2 (output rows i in
[2048h, 2048h+2048)).  Weights replicated; each core holds its batch's
similarity implicitly as resident exp(Sim^T) tiles.

Math (per batch, n/i index valR positions, m/j index valL positions):
    Sim[n,m] = sum_c valR[c,n] valL[c,m]
    func1 = softmax(Sim, axis=1)  -> denominators Z[m]    = sum_n e[n,m]
    func2 = softmax(Sim, axis=2)  -> denominators Zr[n]   = sum_m e[n,m]
    Att1[c,i] = sum_m e[i,m] * valF[c,m] / Z[m]
    Att2[c,i] = sum_m e[i,m] * valF[c,m] / Zr[i]
|Sim| < ~1 for these inputs so the softmax max-subtraction is skipped
(shift-invariant; identical result up to fp rounding) and e = exp(Sim)
directly.  E^T[m, i] is computed tile-by-tile on PE (bf16), exponentiated
on ScalarE (accum_out gives the per-m partial Z for free), and kept
resident in SBUF as bf16 [128, 32, 2048].  The two cores of a batch
AllReduce their partial Z ([128,32] f32, 16KB).  One fused matmul per
i-chunk contracts E^T with [valF/Z | valF | 1] producing Att1, Att2*Zr
and Zr together; gates + broadcasts across partitions are done with two
more small matmuls (a host-built [65,64] block weight computes both gate
dot products pre-broadcast to 32 partitions each).

kernel(**inputs) takes the full unsharded reference inputs and returns
(out_L, out_R) like the reference.
"""

import os
import numpy as np

B, C, N = 4, 32, 4096
NH = N // 2          # per-core i-extent
JT = N // 128        # 32 m-tiles of 128
IC = NH // 512       # 4 i-chunks of 512
ICF = N // 512       # 8 chunks over the full range
N_CORES = 8

_CACHE = {}
LAST_RESULTS = None


def _apply_walrus_wait_patch():
    """This walrus build rejects >1 sync-wait command per instruction
    ("Too many sync wait commands"); Tile attaches several (body insts via
    assign_waits, the tail drain via add_sem_waits).  Split the extras
    onto preceding single-wait nofuse nops on the same engine."""
    import concourse.tile as tile
    from concourse import mybir

    if getattr(tile.TileContext, "_ant_wait_patched", False):
        return

    def _split(tc, ordered):
        nc = tc.nc
        for insts in ordered.values():
            new_list = []
            for inst in insts:
                si = getattr(inst, "sync_info", None)
                if (
                    si is not None
                    and si.on_wait
                    and len(si.on_wait) > 1
                    and not isinstance(inst, mybir.InstEventSemaphore)
                ):
                    waits = list(si.on_wait)
                    for w in waits[:-1]:
                        nop = mybir.InstNoOp(
                            name=nc.get_next_instruction_name(),
                            sync_info=mybir.SyncInfo(on_wait=[w], on_update=[]),
                            bass_nofuse=True,
                            engine=inst.engine,
                        )
                        nc.register_instruction(nop)
                        new_list.append(nop)
                    inst.sync_info = mybir.SyncInfo(
                        on_wait=[waits[-1]], on_update=list(si.on_update)
                    )
                new_list.append(inst)
            insts[:] = new_list

    orig_lower = tile.TileContext._lower_ordered_insts

    def patched_lower(self, ordered):
        _split(self, ordered)
        return orig_lower(self, ordered)

    def patched_drain(self, tick_clock, wait_clock):
        from concourse.tile import ScopedClock

        probe = self.nc.sync.nop(nofuse=True, hint="tail_wait_probe")
        wait_clock.add_sem_waits(
            probe.ins, ScopedClock({None: tick_clock.global_clock})
        )
        si = probe.ins.sync_info
        waits = list(si.on_wait) if si is not None else []
        if len(waits) > 1:
            probe.ins.sync_info = mybir.SyncInfo(
                on_wait=[waits[0]], on_update=list(si.on_update)
            )
            for w in waits[1:]:
                n2 = self.nc.sync.nop(nofuse=True, hint="tail_wait_extra")
                n2.ins.sync_info = mybir.SyncInfo(on_wait=[w], on_update=[])
        self.nc.sync.drain()
        self.nc.all_engine_barrier()
        popped = self.nc._tile_sem_poison_stack.pop()
        assert popped is self._sem_poison
        self.nc.clear_and_free_semaphores(list(self.sems.allocated().values()))
        self.nc.all_engine_barrier()

    tile.TileContext._lower_ordered_insts = patched_lower
    tile.TileContext._drain_and_barrier = patched_drain
    tile.TileContext._ant_wait_patched = True


def _build_program():
    from contextlib import ExitStack

    import concourse.bass as bass
    import concourse.tile as tile
    from concourse import mybir

    _apply_walrus_wait_patch()

    f32 = mybir.dt.float32
    bf16 = mybir.dt.bfloat16
    Exp = mybir.ActivationFunctionType.Exp
    Sigmoid = mybir.ActivationFunctionType.Sigmoid

    nc = bass.Bass(num_devices=N_CORES)

    # ---- external I/O (per-core shard shapes) ----
    D = {}
    for nm in ("xlh", "xll", "xrh", "xrl"):
        D[nm] = nc.dram_tensor(nm, [C, N], f32, kind="ExternalInput")
    for nm in ("xlh_i", "xll_i", "xrh_i", "xrl_i"):
        D[nm] = nc.dram_tensor(nm, [C, NH], f32, kind="ExternalInput")
    for nm in ("wclt", "wcrt", "wcft"):
        D[nm] = nc.dram_tensor(nm, [2 * C, C], f32, kind="ExternalInput")
    for nm in ("wwlt", "wwrt", "wwft"):
        D[nm] = nc.dram_tensor(nm, [C, C], f32, kind="ExternalInput")
    D["gwbc"] = nc.dram_tensor("gwbc", [65, 64], f32, kind="ExternalInput")
    for nm in ("bcl", "bcr", "bcf", "bwl", "bwr"):
        D[nm] = nc.dram_tensor(nm, [C, 1], f32, kind="ExternalInput")
    D["bwf"] = nc.dram_tensor("bwf", [C], f32, kind="ExternalInput")
    D["bg"] = nc.dram_tensor("bg", [64, 1], f32, kind="ExternalInput")
    D["out_l"] = nc.dram_tensor("out_l", [C, NH], f32, kind="ExternalOutput")
    D["out_r"] = nc.dram_tensor("out_r", [C, NH], f32, kind="ExternalOutput")

    with tile.TileContext(nc) as tc, ExitStack() as ctx:
        const = ctx.enter_context(tc.tile_pool(name="const", bufs=1))
        persist = ctx.enter_context(tc.tile_pool(name="persist", bufs=1))
        dram = ctx.enter_context(tc.tile_pool(name="dram", bufs=1, space="DRAM"))

        cs = {}
        for nm in (
            "wclt", "wcrt", "wcft", "wwlt", "wwrt", "wwft",
            "gwbc", "bcl", "bcr", "bcf", "bwl", "bwr", "bg",
        ):
            t = const.tile(list(D[nm].shape), f32, name=f"c_{nm}")
            nc.sync.dma_start(out=t, in_=D[nm][:, :])
            cs[nm] = t
        bwf_bc = const.tile([128, C], f32, name="bwf_bc")
        nc.gpsimd.dma_start(out=bwf_bc, in_=D["bwf"][:].partition_broadcast(128))
        onesb = const.tile([128, 64], f32, name="onesb")
        nc.vector.memset(onesb, 1.0)

        # persistent state
        ET = persist.tile([128, JT, NH], bf16, name="ET")          # E^T[m, i]
        valLb = persist.tile([C, N], bf16, name="valLb")
        valRb = persist.tile([C, NH], bf16, name="valRb")
        combo = persist.tile([128, JT, 65], bf16, name="combo")
        xres = persist.tile([64, NH], f32, name="xres")            # [xL_i; xR_i]
        zcols = persist.tile([128, JT], f32, name="zcols")
        zfull = persist.tile([128, JT], f32, name="zfull")
        rz = persist.tile([128, JT], f32, name="rz")

        z_in = dram.tile([128, JT], f32, name="z_in")
        z_out = dram.tile([128, JT], f32, name="z_out")

        # ---------------- phase 0: convs + vals ----------------
        with tc.tile_pool(name="p0s", bufs=3) as p0s, tc.tile_pool(
            name="p0p", bufs=4, space="PSUM"
        ) as p0p:
            # full-range pass: valL (all m) and valF^T (all m) into combo
            for ch in range(ICF):
                sl = slice(ch * 512, (ch + 1) * 512)

                xcat = p0s.tile([2 * C, 512], f32, tag="xcat")
                nc.sync.dma_start(out=xcat[0:C, :], in_=D["xlh"][:, sl])
                nc.sync.dma_start(out=xcat[C:, :], in_=D["xll"][:, sl])
                psl = p0p.tile([C, 512], f32, tag="convps")
                nc.tensor.matmul(psl, cs["wclt"], xcat, start=True, stop=True)
                xLs = p0s.tile([C, 512], f32, tag="xls")
                nc.vector.tensor_scalar_add(out=xLs, in0=psl, scalar1=cs["bcl"])

                xcat2 = p0s.tile([2 * C, 512], f32, tag="xcat")
                nc.sync.dma_start(out=xcat2[0:C, :], in_=D["xrh"][:, sl])
                nc.sync.dma_start(out=xcat2[C:, :], in_=D["xrl"][:, sl])
                psr = p0p.tile([C, 512], f32, tag="convps")
                nc.tensor.matmul(psr, cs["wcrt"], xcat2, start=True, stop=True)
                xRs = p0s.tile([C, 512], f32, tag="xrs")
                nc.vector.tensor_scalar_add(out=xRs, in0=psr, scalar1=cs["bcr"])

                psv = p0p.tile([C, 512], f32, tag="convps")
                nc.tensor.matmul(psv, cs["wwlt"], xLs, start=True, stop=True)
                nc.vector.tensor_scalar_add(
                    out=valLb[:, sl], in0=psv, scalar1=cs["bwl"]
                )

                xcatf = p0s.tile([2 * C, 512], f32, tag="xcat")
                nc.vector.tensor_copy(out=xcatf[0:C, :], in_=xLs)
                nc.gpsimd.dma_start(out=xcatf[C:, :], in_=xRs)
                psf = p0p.tile([C, 512], f32, tag="convps")
                nc.tensor.matmul(psf, cs["wcft"], xcatf, start=True, stop=True)
                xFs = p0s.tile([C, 512], f32, tag="xfs")
                nc.vector.tensor_scalar_add(out=xFs, in0=psf, scalar1=cs["bcf"])

                for q in range(4):
                    jt = ch * 4 + q
                    pft = p0p.tile([128, C], f32, tag="vftps")
                    nc.tensor.matmul(
                        pft,
                        xFs[:, q * 128 : (q + 1) * 128],
                        cs["wwft"],
                        start=True,
                        stop=True,
                    )
                    nc.vector.tensor_add(
                        out=combo[:, jt, C : 2 * C], in0=pft, in1=bwf_bc
                    )

            # i-half pass (host-sliced inputs): residuals + valR
            for ch in range(IC):
                sl = slice(ch * 512, (ch + 1) * 512)

                xcat = p0s.tile([2 * C, 512], f32, tag="xcat")
                nc.sync.dma_start(out=xcat[0:C, :], in_=D["xlh_i"][:, sl])
                nc.sync.dma_start(out=xcat[C:, :], in_=D["xll_i"][:, sl])
                psl = p0p.tile([C, 512], f32, tag="convps")
                nc.tensor.matmul(psl, cs["wclt"], xcat, start=True, stop=True)
                nc.vector.tensor_scalar_add(
                    out=xres[0:C, sl], in0=psl, scalar1=cs["bcl"]
                )

                xcat2 = p0s.tile([2 * C, 512], f32, tag="xcat")
                nc.sync.dma_start(out=xcat2[0:C, :], in_=D["xrh_i"][:, sl])
                nc.sync.dma_start(out=xcat2[C:, :], in_=D["xrl_i"][:, sl])
                psr = p0p.tile([C, 512], f32, tag="convps")
                nc.tensor.matmul(psr, cs["wcrt"], xcat2, start=True, stop=True)
                xRs = p0s.tile([C, 512], f32, tag="xrs")
                nc.vector.tensor_scalar_add(out=xRs, in0=psr, scalar1=cs["bcr"])
                nc.gpsimd.dma_start(out=xres[C:, sl], in_=xRs)

                psv = p0p.tile([C, 512], f32, tag="convps")
                nc.tensor.matmul(psv, cs["wwrt"], xRs, start=True, stop=True)
                nc.vector.tensor_scalar_add(
                    out=valRb[:, sl], in0=psv, scalar1=cs["bwr"]
                )

        nc.vector.memset(combo[:, :, 64:65], 1.0)

        # ---------------- phase 1: E^T tiles + partial Z ----------------
        with tc.tile_pool(name="p1p", bufs=2, space="PSUM") as p1p:
            for jt in range(JT):
                ps = p1p.tile([128, NH], f32, tag="simps")
                for ic in range(IC):
                    nc.tensor.matmul(
                        ps[:, ic * 512 : (ic + 1) * 512],
                        valLb[:, jt * 128 : (jt + 1) * 128],
                        valRb[:, ic * 512 : (ic + 1) * 512],
                        start=True,
                        stop=True,
                    )
                nc.scalar.activation(
                    out=ET[:, jt, :],
                    in_=ps,
                    func=Exp,
                    accum_out=zcols[:, jt : jt + 1],
                )

        # ---------------- Z exchange within the batch pair ----------------
        nc.sync.dma_start(out=z_in, in_=zcols)
        nc.gpsimd.collective_compute(
            "AllReduce",
            mybir.AluOpType.add,
            replica_groups=[[0, 1], [2, 3], [4, 5], [6, 7]],
            ins=[z_in[:, :]],
            outs=[z_out[:, :]],
        )
        nc.sync.dma_start(out=zfull, in_=z_out)
        nc.vector.reciprocal(out=rz, in_=zfull)
        for jt in range(JT):
            nc.vector.tensor_scalar_mul(
                out=combo[:, jt, 0:C],
                in0=combo[:, jt, C : 2 * C],
                scalar1=rz[:, jt : jt + 1],
            )

        # ---------------- phase 2: attention + gates + residual ----------------
        with tc.tile_pool(name="p2p", bufs=2, space="PSUM") as p2p, tc.tile_pool(
            name="p2s", bufs=2
        ) as p2s:
            for ic in range(IC):
                isl = slice(ic * 512, (ic + 1) * 512)
                att_ps = p2p.tile([65, 512], f32, tag="attps")
                for jt in range(JT):
                    nc.tensor.matmul(
                        att_ps,
                        combo[:, jt, :],
                        ET[:, jt, isl],
                        start=(jt == 0),
                        stop=(jt == JT - 1),
                    )
                att_sb = p2s.tile([65, 512], f32, tag="attsb")
                nc.vector.tensor_copy(out=att_sb, in_=att_ps)

                # gates pre-broadcast: rows 0-31 <- WgL . Att1, rows 32-63 <- WgR . Att2u
                bcg = p2p.tile([64, 512], f32, tag="bcgps")
                nc.tensor.matmul(bcg, cs["gwbc"], att_sb, start=True, stop=True)
                # Zr broadcast to rows 32-63
                bcz = p2p.tile([64, 512], f32, tag="bczps")
                nc.tensor.matmul(
                    bcz[32:64, :],
                    onesb[64:65, 32:64],
                    att_sb[64:65, :],
                    start=True,
                    stop=True,
                    tile_position=(64, 32),
                )
                rcp = p2s.tile([64, 512], f32, tag="rcp")
                nc.vector.reciprocal(out=rcp[32:64, :], in_=bcz[32:64, :])

                gpre = p2s.tile([64, 512], f32, tag="gpre")
                nc.vector.tensor_copy(out=gpre[0:32, :], in_=bcg[0:32, :])
                nc.vector.tensor_mul(
                    out=gpre[32:64, :], in0=bcg[32:64, :], in1=rcp[32:64, :]
                )
                sig = p2s.tile([64, 512], f32, tag="sig")
                nc.scalar.activation(
                    out=sig, in_=gpre, func=Sigmoid, bias=cs["bg"], scale=1.0
                )
                # fold 1/Zr into gate2 lane so one multiply normalizes Att2
                nc.vector.tensor_mul(
                    out=sig[32:64, :], in0=sig[32:64, :], in1=rcp[32:64, :]
                )
                outt = p2s.tile([64, 512], f32, tag="outt")
                nc.vector.tensor_mul(out=outt, in0=att_sb[0:64, :], in1=sig)
                nc.vector.tensor_add(out=outt, in0=outt, in1=xres[:, isl])

                nc.sync.dma_start(out=D["out_l"][:, isl], in_=outt[0:C, :])
                nc.sync.dma_start(out=D["out_r"][:, isl], in_=outt[C:, :])

    return nc


def _host_inputs(inputs):
    """Build the per-core input maps from the full reference inputs."""
    f = np.float32
    x = {
        nm: np.ascontiguousarray(
            np.asarray(inputs[nm], dtype=f).reshape(B, C, N)
        )
        for nm in ("xlh", "xll", "xrh", "xrl")
    }
    WcL, WcR, WcF = (np.asarray(inputs[k], f) for k in ("WcL", "WcR", "WcF"))
    WwL, WwR, WwF = (np.asarray(inputs[k], f) for k in ("WwL", "WwR", "WwF"))
    WgL, WgR = np.asarray(inputs["WgL"], f), np.asarray(inputs["WgR"], f)
    bcL, bcR, bcF = (np.asarray(inputs[k], f) for k in ("bcL", "bcR", "bcF"))
    bwL, bwR, bwF = (np.asarray(inputs[k], f) for k in ("bwL", "bwR", "bwF"))
    bgL, bgR = np.asarray(inputs["bgL"], f), np.asarray(inputs["bgR"], f)

    gwbc = np.zeros((65, 64), f)
    gwbc[0:32, 0:32] = WgL[0][:, None]     # WgL . Att1 -> rows 0-31
    gwbc[32:64, 32:64] = WgR[0][:, None]   # WgR . Att2u -> rows 32-63
    bg = np.concatenate([np.full(32, bgL[0], f), np.full(32, bgR[0], f)])

    shared = {
        "wclt": np.ascontiguousarray(WcL.T),
        "wcrt": np.ascontiguousarray(WcR.T),
        "wcft": np.ascontiguousarray(WcF.T),
        "wwlt": np.ascontiguousarray(WwL.T),
        "wwrt": np.ascontiguousarray(WwR.T),
        "wwft": np.ascontiguousarray(WwF.T),
        "gwbc": gwbc,
        "bcl": bcL.reshape(C, 1),
        "bcr": bcR.reshape(C, 1),
        "bcf": bcF.reshape(C, 1),
        "bwl": bwL.reshape(C, 1),
        "bwr": bwR.reshape(C, 1),
        "bwf": np.ascontiguousarray(bwF),
        "bg": bg.reshape(64, 1),
    }

    in_maps = []
    for k in range(N_CORES):
        b, h = k // 2, k % 2
        isl = slice(h * NH, (h + 1) * NH)
        m = dict(shared)
        for nm in ("xlh", "xll", "xrh", "xrl"):
            m[nm] = x[nm][b]
            m[nm + "_i"] = np.ascontiguousarray(x[nm][b][:, isl])
        in_maps.append(m)
    return in_maps


def kernel(**inputs):
    global LAST_RESULTS
    from concourse.bass_utils import run_bass_kernel_spmd

    if "nc" not in _CACHE:
        _CACHE["nc"] = _build_program()
    nc = _CACHE["nc"]

    in_maps = _host_inputs(inputs)
    res = run_bass_kernel_spmd(nc, in_maps, core_ids=list(range(N_CORES)))
    LAST_RESULTS = res

    out_L = np.empty((B, C, N), np.float32)
    out_R = np.empty((B, C, N), np.float32)
    for k in range(N_CORES):
        b, h = k // 2, k % 2
        isl = slice(h * NH, (h + 1) * NH)
        out_L[b, :, isl] = res.results[k]["out_l"]
        out_R[b, :, isl] = res.results[k]["out_r"]
    return (
        out_L.reshape(B, C, 64, 64),
        out_R.reshape(B, C, 64, 64),
    )
